# revision 6
# baseline (speedup 1.0000x reference)
"""Trainium2 Bass kernel for nn_CasDOSeqModel.

Pipeline (per batch row):
  z_clean = LayerNorm(bert_emb @ proj_W + proj_b) * ln_g + ln_b
  z_aug   = silu((z_clean + sigma*eps) @ den_W1 + den_b1) @ den_W2 + den_b2
  diff_loss = mean((z_aug - z_clean)^2)
  Euler scan, T steps: z <- z + dt * (tanh(tanh(z@W1+b1)@W2+b2)@W3 + b3)
  out_t = z_t @ dec_W + dec_b  (2 channels per step; T+1 steps incl. z_aug)

Sharding: pure data parallel, batch 4096 -> 8 cores x 512.

Key kernel-level transforms:
  * The scan state is kept as H = z @ W1 in a persistent PSUM accumulator;
    each step the tensor engine accumulates H += tanh2 @ (dt*W3@W1), so no
    vector-engine state update is needed. Bias terms are linear in t and are
    folded into a per-step activation-bias table.
  * The decode is folded into the scan: per step g_t = tanh2 @ (dt*W3@dec_W)
    (+ dt*b3@dec_W) is streamed to DRAM; out_t is recovered afterwards as
    out_0 + cumsum(g) via upper-triangular-matrix matmuls in batch-major
    layout, so the (B, T+1, 64) trajectory is never materialized.
  * Scan matmuls run in float32r (full-rate fp32 on the PE array).
"""

import os
import numpy as np

import concourse.bass as bass
import concourse.mybir as mybir
import concourse.tile as tile

F32 = mybir.dt.float32
F32R = mybir.dt.float32r
AF = mybir.ActivationFunctionType
ALU = mybir.AluOpType

N_CORES = 8
B = 4096
BD = 768      # bert dim
D = 64        # z dim
U = 128       # ode units
T = 512       # scan steps
BS = B // N_CORES   # 512 rows per core
NBT = BS // 128     # 4 batch tiles of 128
NKT = BD // 128     # 6 contraction tiles for the projection
GEN = 32            # scan steps per G-spill generation
NGEN = T // GEN
CUM = 64            # cumsum block size in post-processing
NCUM = T // CUM
LN_EPS = 1e-5

SCAN_F32R = True    # float32r for the scan matmuls (4x faster than fp32)
DEBUG = os.environ.get("KERNEL_DEBUG", "0") == "1"

# Set by run() when trace is requested via KERNEL_TRACE=1 (used by test.py).
last_results = None

# ----------------------------------------------------------------------------
# Workarounds for this walrus build rejecting >1 sem wait (+1 update) per
# instruction: split Tile's tail-drain waits over nofuse NoOps, and hoist
# excess waits in the final BIR onto injected NoOps.
# ----------------------------------------------------------------------------
_N_PROCS = 27
_patched = False


def _patched_drain_and_barrier(self, tick_clock, wait_clock):
    from concourse.vector_clock import ScopedClock, VectorClock

    nc = self.nc
    gc = tick_clock.global_clock
    for p in range(_N_PROCS):
        if not gc[p]:
            continue
        vals = [gc[q] if q == p else 0 for q in range(_N_PROCS)]
        d = nc.sync.nop(nofuse=True)
        wait_clock.add_sem_waits(d.ins, ScopedClock({None: VectorClock(vals)}))
    nc.sync.drain()

    nc.all_engine_barrier()
    assert self.sems is not None
    popped = nc._tile_sem_poison_stack.pop()
    assert popped is self._sem_poison
    nc.clear_and_free_semaphores(list(self.sems.allocated().values()))
    nc.all_engine_barrier()


def _split_excess_waits(bir_json: bytes) -> bytes:
    import json

    m = json.loads(bir_json)
    for f in m.get("functions", []):
        for blk in f.get("blocks", []):
            out = []
            for inst in blk.get("instructions", []):
                si = inst.get("sync_info") or {}
                waits = si.get("on_wait") or []
                updates = si.get("on_update") or []
                allowed = max(0, min(1, 2 - len(updates)))
                if len(waits) > allowed:
                    excess = waits[: len(waits) - allowed]
                    si["on_wait"] = waits[len(waits) - allowed:]
                    for ci, w in enumerate(excess):
                        out.append({
                            "debug": inst.get("debug", 0),
                            "engine": inst["engine"],
                            "ins": [],
                            "outs": [],
                            "name": f"{inst['name']}-sw{ci}",
                            "opcode": "NoOp",
                            "sync_info": {"on_update": [], "on_wait": [w]},
                        })
                out.append(inst)
            blk["instructions"] = out
    return json.dumps(m).encode()


def _apply_patches():
    global _patched
    if _patched:
        return
    _patched = True

    tile.TileContext._drain_and_barrier = _patched_drain_and_barrier

    import concourse.bass_utils as bu
    import concourse.bass2jax as b2j

    orig = bu.compile_bir_kernel

    def patched_compile_bir_kernel(bir_json, tmpdir, neff_name="file.neff"):
        return orig(_split_excess_waits(bir_json), tmpdir, neff_name)

    bu.compile_bir_kernel = patched_compile_bir_kernel
    b2j.compile_bir_kernel = patched_compile_bir_kernel


# ----------------------------------------------------------------------------
# Kernel build
# ----------------------------------------------------------------------------

def _build(sigma: float):
    nc = bass.Bass()
    dt_ = 1.0 / T

    # per-core shard inputs
    bert = nc.dram_tensor("bert", [BS, BD], F32, kind="ExternalInput")
    eps = nc.dram_tensor("eps", [BS, D], F32, kind="ExternalInput")
    # replicated params (host-packed)
    projw = nc.dram_tensor("projw", [NKT, 128, D], F32, kind="ExternalInput")
    projb = nc.dram_tensor("projb", [D, 1], F32, kind="ExternalInput")
    lng = nc.dram_tensor("lng", [D, 1], F32, kind="ExternalInput")
    lnb = nc.dram_tensor("lnb", [D, 1], F32, kind="ExternalInput")
    denw1 = nc.dram_tensor("denw1", [D, U], F32, kind="ExternalInput")
    denb1 = nc.dram_tensor("denb1", [U, 1], F32, kind="ExternalInput")
    denw2 = nc.dram_tensor("denw2", [U, D], F32, kind="ExternalInput")
    denb2 = nc.dram_tensor("denb2", [D, 1], F32, kind="ExternalInput")
    w1 = nc.dram_tensor("w1", [D, U], F32, kind="ExternalInput")
    w2 = nc.dram_tensor("w2", [U, U], F32, kind="ExternalInput")
    w31 = nc.dram_tensor("w31", [U, U], F32, kind="ExternalInput")      # dt*W3@W1
    w3g = nc.dram_tensor("w3g", [U, 2], F32, kind="ExternalInput")      # dt*W3@dec_W
    bias1 = nc.dram_tensor("bias1", [U, T], F32, kind="ExternalInput")  # b1 + t*dt*(b3@W1)
    b2c = nc.dram_tensor("b2c", [U, 1], F32, kind="ExternalInput")
    corr = nc.dram_tensor("corr", [2, 1], F32, kind="ExternalInput")    # dt*(b3@dec_W)
    decw = nc.dram_tensor("decw", [D, 2], F32, kind="ExternalInput")
    decb = nc.dram_tensor("decb", [2, 1], F32, kind="ExternalInput")
    utri = nc.dram_tensor("utri", [CUM, CUM], F32, kind="ExternalInput")
    ident = nc.dram_tensor("ident", [128, 128], F32, kind="ExternalInput")
    invd = nc.dram_tensor("invd", [D, 1], F32, kind="ExternalInput")    # 1/D
    onesd = nc.dram_tensor("onesd", [D, 1], F32, kind="ExternalInput")  # 1.0
    ones1 = nc.dram_tensor("ones1", [1, D], F32, kind="ExternalInput")
    epsc = nc.dram_tensor("epsc", [1, 1], F32, kind="ExternalInput")    # LN_EPS
    sigc = nc.dram_tensor("sigc", [D, 1], F32, kind="ExternalInput")    # sigma  # 1.0 row

    # outputs
    pred = nc.dram_tensor("pred", [BS, T + 1], F32, kind="ExternalOutput")
    stp = nc.dram_tensor("stp", [BS, T + 1], F32, kind="ExternalOutput")
    loss = nc.dram_tensor("loss", [1, 1], F32, kind="ExternalOutput")
    if DEBUG:
        dbg_eps = nc.dram_tensor("dbg_eps", [D, BS], F32, kind="ExternalOutput")
        dbg_zc = nc.dram_tensor("dbg_zc", [D, BS], F32, kind="ExternalOutput")
        dbg_za = nc.dram_tensor("dbg_za", [D, BS], F32, kind="ExternalOutput")
        dbg_o0 = nc.dram_tensor("dbg_o0", [2, BS], F32, kind="ExternalOutput")
        dbg_g = nc.dram_tensor("dbg_g", [2, T, BS], F32, kind="ExternalOutput")
        dbg_h = nc.dram_tensor("dbg_h", [D, BS], F32, kind="ExternalOutput")
        dbg_bt = nc.dram_tensor("dbg_bt", [128, NKT * BS], F32, kind="ExternalOutput")
        dbg_ms = nc.dram_tensor("dbg_ms", [2, BS], F32, kind="ExternalOutput")

    sdt = F32R if SCAN_F32R else F32

    with tile.TileContext(nc) as tc:
        with (
            tc.tile_pool(name="cst", bufs=1) as cst,
            tc.tile_pool(name="dr", bufs=1, space="DRAM") as dr,
            tc.tile_pool(name="hp", bufs=1, space="PSUM") as hp,
        ):
            gdram = dr.tile([2, T, BS], F32)

            # ---- load constants
            def cload(name, dram_ap, shape, dtype=F32):
                t_ = cst.tile(shape, dtype, tag=name)
                nc.sync.dma_start(out=t_[:], in_=dram_ap)
                return t_

            projw_sb = cload("projw", projw.rearrange("k p m -> p k m"), [128, NKT, D])
            projb_sb = cload("projb", projb[:, :], [D, 1])
            lng_sb = cload("lng", lng[:, :], [D, 1])
            lnb_sb = cload("lnb", lnb[:, :], [D, 1])
            denw1_sb = cload("denw1", denw1[:, :], [D, U])
            denb1_sb = cload("denb1", denb1[:, :], [U, 1])
            denw2_sb = cload("denw2", denw2[:, :], [U, D])
            denb2_sb = cload("denb2", denb2[:, :], [D, 1])
            w1_sb = cload("w1", w1[:, :], [D, U])
            w2_sb = cload("w2", w2[:, :], [U, U])
            w31_sb = cload("w31", w31[:, :], [U, U])
            w3g_sb = cload("w3g", w3g[:, :], [U, 2])
            bias1_sb = cload("bias1", bias1[:, :], [U, T])
            b2_sb = cload("b2c", b2c[:, :], [U, 1])
            corr_sb = cload("corr", corr[:, :], [2, 1])
            decw_sb = cload("decw", decw[:, :], [D, 2])
            decb_sb = cload("decb", decb[:, :], [2, 1])
            utri_sb = cload("utri", utri[:, :], [CUM, CUM])
            id_sb = cload("ident", ident[:, :], [128, 128])
            invd_sb = cload("invd", invd[:, :], [D, 1])
            onesd_sb = cload("onesd", onesd[:, :], [D, 1])
            ones1_sb = cload("ones1", ones1[:, :], [1, D])
            epsc_sb = cload("epsc", epsc[:, :], [1, 1])
            sigc_sb = cload("sigc", sigc[:, :], [D, 1])

            # float32r-rounded scan weights
            if SCAN_F32R:
                w1_r = cst.tile([D, U], F32R, tag="w1r")
                nc.vector.tensor_copy(w1_r[:], w1_sb[:])
                w2_r = cst.tile([U, U], F32R, tag="w2r")
                nc.vector.tensor_copy(w2_r[:], w2_sb[:])
                w31_r = cst.tile([U, U], F32R, tag="w31r")
                nc.vector.tensor_copy(w31_r[:], w31_sb[:])
                w3g_r = cst.tile([U, 2], F32R, tag="w3gr")
                nc.vector.tensor_copy(w3g_r[:], w3g_sb[:])
            else:
                w1_r, w2_r, w31_r, w3g_r = w1_sb, w2_sb, w31_sb, w3g_sb

            zaug_r = cst.tile([D, BS], sdt, tag="zaugr")
            out0T = cst.tile([2, BS], F32, tag="out0T")
            H = hp.tile([U, BS], F32)

            # ================= pre-stage =================
            with (
                tc.tile_pool(name="pre", bufs=2) as pre,
                tc.tile_pool(name="pps", bufs=3, space="PSUM") as pps,
            ):
                # bert (BS, BD) -> feature-major bertT (128, NKT, BS) via PE transposes
                bert_nat = pre.tile([128, NBT, BD], F32, tag="bertnat")
                nc.sync.dma_start(
                    out=bert_nat[:], in_=bert.rearrange("(i p) f -> p i f", p=128)
                )
                bertT = pre.tile([128, NKT, BS], F32, tag="bertT")
                for i in range(NBT):
                    for k in range(NKT):
                        tp = pps.tile([128, 128], F32, tag="pp")
                        nc.tensor.transpose(
                            tp[:], bert_nat[:, i, 128 * k:128 * (k + 1)], id_sb[:]
                        )
                        nc.vector.tensor_copy(
                            bertT[:, k, 128 * i:128 * (i + 1)], tp[:]
                        )

                # projection: hT (D, BS)
                ph = pps.tile([D, BS], F32, tag="pp")
                for k in range(NKT):
                    nc.tensor.matmul(
                        ph[:], projw_sb[:, k, :], bertT[:, k, :],
                        start=(k == 0), stop=(k == NKT - 1),
                    )
                hT = pre.tile([D, BS], F32, tag="hT")
                nc.scalar.activation(out=hT[:], in_=ph[:], func=AF.Identity,
                                     bias=projb_sb[:])

                # layernorm stats via matmul reductions over the partition axis
                hsq = pre.tile([D, BS], F32, tag="hsq")
                nc.vector.tensor_mul(hsq[:], hT[:], hT[:])
                pm = pps.tile([1, BS], F32, tag="pp")
                nc.tensor.matmul(pm[:], invd_sb[:], hT[:], start=True, stop=True)
                pmsq = pps.tile([1, BS], F32, tag="pp")
                nc.tensor.matmul(pmsq[:], invd_sb[:], hsq[:], start=True, stop=True)
                mean_s = pre.tile([1, BS], F32, tag="mean_s")
                nc.vector.tensor_copy(mean_s[:], pm[:])
                msq_s = pre.tile([1, BS], F32, tag="msq_s")
                nc.vector.tensor_mul(msq_s[:], mean_s[:], mean_s[:])  # mean^2
                var_s = pre.tile([1, BS], F32, tag="var_s")
                nc.vector.tensor_sub(var_s[:], pmsq[:], msq_s[:])
                std_s = pre.tile([1, BS], F32, tag="std_s")
                nc.scalar.activation(out=std_s[:], in_=var_s[:], func=AF.Sqrt,
                                     bias=epsc_sb[:])
                rstd_s = pre.tile([1, BS], F32, tag="rstd_s")
                nc.vector.reciprocal(rstd_s[:], std_s[:])

                # broadcast mean/rstd across D partitions with K=1 matmuls
                pmb = pps.tile([D, BS], F32, tag="pp")
                nc.tensor.matmul(pmb[:], ones1_sb[:], mean_s[:], start=True, stop=True)
                prb = pps.tile([D, BS], F32, tag="pp")
                nc.tensor.matmul(prb[:], ones1_sb[:], rstd_s[:], start=True, stop=True)

                zc = pre.tile([D, BS], F32, tag="zc")
                nc.vector.tensor_sub(zc[:], hT[:], pmb[:])
                zc2 = pre.tile([D, BS], F32, tag="zc2")
                nc.vector.tensor_mul(zc2[:], zc[:], prb[:])
                zcT = pre.tile([D, BS], F32, tag="zcT")
                nc.vector.tensor_scalar(
                    out=zcT[:], in0=zc2[:], scalar1=lng_sb[:], scalar2=lnb_sb[:],
                    op0=ALU.mult, op1=ALU.add,
                )

                # eps -> feature-major, z_noisy = zc + sigma*eps
                eps_nat = pre.tile([128, NBT, D], F32, tag="epsnat")
                nc.sync.dma_start(
                    out=eps_nat[:], in_=eps.rearrange("(i p) f -> p i f", p=128)
                )
                epsT = pre.tile([D, BS], F32, tag="epsT")
                for i in range(NBT):
                    tp = pps.tile([D, 128], F32, tag="pp")
                    nc.tensor.transpose(tp[:], eps_nat[:, i, :], id_sb[:])
                    nc.vector.tensor_copy(epsT[:, 128 * i:128 * (i + 1)], tp[:])
                seps = pre.tile([D, BS], F32, tag="seps")
                nc.scalar.activation(out=seps[:], in_=epsT[:], func=AF.Identity,
                                     scale=sigc_sb[:])
                znT = pre.tile([D, BS], F32, tag="znT")
                nc.vector.tensor_add(znT[:], zcT[:], seps[:])

                # denoiser MLP
                pu1 = pps.tile([U, BS], F32, tag="pp")
                nc.tensor.matmul(pu1[:], denw1_sb[:], znT[:], start=True, stop=True)
                s1 = pre.tile([U, BS], F32, tag="s1")
                nc.scalar.activation(out=s1[:], in_=pu1[:], func=AF.Silu,
                                     bias=denb1_sb[:])
                pu2 = pps.tile([D, BS], F32, tag="pp")
                nc.tensor.matmul(pu2[:], denw2_sb[:], s1[:], start=True, stop=True)
                zaugT = pre.tile([D, BS], F32, tag="zaugT")
                nc.vector.tensor_scalar(
                    out=zaugT[:], in0=pu2[:], scalar1=denb2_sb[:], scalar2=None,
                    op0=ALU.add,
                )
                nc.vector.tensor_copy(zaug_r[:], zaugT[:])

                # diff loss: sum((z_aug - z_clean)^2) -> scalar
                dly = pre.tile([D, BS], F32, tag="dly")
                nc.vector.tensor_sub(dly[:], zaugT[:], zcT[:])
                dsq = pre.tile([D, BS], F32, tag="dsq")
                losscol = pre.tile([D, 1], F32, tag="losscol")
                nc.scalar.activation(out=dsq[:], in_=dly[:], func=AF.Square,
                                     accum_out=losscol[:])
                pl = pps.tile([1, 1], F32, tag="pp")
                nc.tensor.matmul(pl[:], losscol[:], onesd_sb[:], start=True, stop=True)
                loss_s = pre.tile([1, 1], F32, tag="loss_s")
                nc.vector.tensor_copy(loss_s[:], pl[:])
                nc.sync.dma_start(out=loss[:, :], in_=loss_s[:])

                # out_0 = z_aug @ dec_W + dec_b   (feature-major (2, BS))
                po0 = pps.tile([2, BS], F32, tag="pp")
                nc.tensor.matmul(po0[:], decw_sb[:], zaugT[:], start=True, stop=True)
                nc.scalar.activation(out=out0T[:], in_=po0[:], func=AF.Identity,
                                     bias=decb_sb[:])

                if DEBUG:
                    nc.sync.dma_start(out=dbg_h[:, :], in_=hT[:])
                    nc.sync.dma_start(out=dbg_bt[:, :], in_=bertT[:].rearrange("p k b -> p (k b)"))
                    nc.sync.dma_start(out=dbg_ms[0:1, :], in_=mean_s[:])
                    nc.sync.dma_start(out=dbg_ms[1:2, :], in_=rstd_s[:])
                    nc.sync.dma_start(out=dbg_eps[:, :], in_=epsT[:])
                    nc.sync.dma_start(out=dbg_zc[:, :], in_=zcT[:])
                    nc.sync.dma_start(out=dbg_za[:, :], in_=zaugT[:])
                    nc.sync.dma_start(out=dbg_o0[:, :], in_=out0T[:])

                # H init: z_aug @ W1 (persistent PSUM accumulator)
                nc.tensor.matmul(H[:], w1_r[:], zaug_r[:], start=True, stop=False,
                                 skip_group_check=True)

            # ================= scan =================
            with (
                tc.tile_pool(name="h1p", bufs=2) as h1p,
                tc.tile_pool(name="h2p", bufs=2) as h2p,
                tc.tile_pool(name="gwp", bufs=2) as gwp,
                tc.tile_pool(name="p2p", bufs=2, space="PSUM") as p2p,
                tc.tile_pool(name="pgp", bufs=2, space="PSUM") as pgp,
            ):
                gw = None
                for t in range(T):
                    h1 = h1p.tile([U, BS], sdt, tag="h1")
                    nc.scalar.activation(out=h1[:], in_=H[:], func=AF.Tanh,
                                         bias=bias1_sb[:, t:t + 1])
                    p2 = p2p.tile([U, BS], F32, tag="p2")
                    nc.tensor.matmul(p2[:], w2_r[:], h1[:], start=True, stop=True)
                    h2 = h2p.tile([U, BS], sdt, tag="h2")
                    nc.scalar.activation(out=h2[:], in_=p2[:], func=AF.Tanh,
                                         bias=b2_sb[:])
                    nc.tensor.matmul(H[:], w31_r[:], h2[:], start=False,
                                     stop=(t == T - 1), skip_group_check=True)
                    pg = pgp.tile([2, BS], F32, tag="pg")
                    nc.tensor.matmul(pg[:], w3g_r[:], h2[:], start=True, stop=True)
                    g = t % GEN
                    if g == 0:
                        gw = gwp.tile([2, GEN * BS], F32, tag="gw")
                    nc.vector.tensor_scalar(
                        out=gw[:, BS * g:BS * (g + 1)], in0=pg[:],
                        scalar1=corr_sb[:], scalar2=None, op0=ALU.add,
                    )
                    if g == GEN - 1:
                        gen = t // GEN
                        nc.sync.dma_start(
                            out=gdram[:, GEN * gen:GEN * (gen + 1), :],
                            in_=gw[:].rearrange("c (m b) -> c m b", m=GEN),
                        )
                        if DEBUG:
                            nc.sync.dma_start(
                                out=dbg_g[:, GEN * gen:GEN * (gen + 1), :],
                                in_=gw[:].rearrange("c (m b) -> c m b", m=GEN),
                            )

            # ================= post: out = out0 + cumsum(g) =================
            with (
                tc.tile_pool(name="lgp", bufs=3) as lgp,
                tc.tile_pool(name="op", bufs=3) as op_,
                tc.tile_pool(name="pcp", bufs=3, space="PSUM") as pcp,
            ):
                for i in range(NBT):
                    o0t = pcp.tile([128, 2], F32, tag="o0t")
                    nc.tensor.transpose(
                        o0t[:], out0T[:, 128 * i:128 * (i + 1)], id_sb[0:2, 0:2]
                    )
                    for c in range(2):
                        ot = op_.tile([128, T + 1], F32, tag="ot")
                        nc.vector.tensor_copy(ot[:, 0:1], o0t[:, c:c + 1])
                        for k in range(NCUM):
                            lg = lgp.tile([CUM, 128], F32, tag="lg")
                            nc.sync.dma_start(
                                out=lg[:],
                                in_=gdram[c, CUM * k:CUM * (k + 1),
                                          128 * i:128 * (i + 1)],
                            )
                            pc = pcp.tile([128, CUM], F32, tag="pc")
                            nc.tensor.matmul(pc[:], lg[:], utri_sb[:],
                                             start=True, stop=True)
                            nc.vector.tensor_scalar(
                                out=ot[:, 1 + CUM * k:1 + CUM * (k + 1)],
                                in0=pc[:],
                                scalar1=ot[:, CUM * k:CUM * k + 1],
                                scalar2=None, op0=ALU.add,
                            )
                        dst = pred if c == 0 else stp
                        nc.sync.dma_start(
                            out=dst[128 * i:128 * (i + 1), :], in_=ot[:]
                        )

    return nc


def kernel(**inputs):
    global last_results
    _apply_patches()

    inp = {k: np.asarray(v) for k, v in inputs.items()}
    bert_emb = inp["bert_emb"].astype(np.float32)
    eps = inp["eps"].astype(np.float32)
    Tdim = inp["times"].shape[1]
    assert Tdim == T, f"kernel compiled for T={T}, got {Tdim}"
    assert bert_emb.shape == (B, BD)

    f32 = np.float32
    dt_ = f32(1.0 / T)
    proj_W = inp["proj_W"].astype(f32)
    proj_b = inp["proj_b"].astype(f32)
    ln_g = inp["ln_g"].astype(f32)
    ln_b = inp["ln_b"].astype(f32)
    log_noise_std = f32(inp["log_noise_std"])
    den_W1 = inp["den_W1"].astype(f32)
    den_b1 = inp["den_b1"].astype(f32)
    den_W2 = inp["den_W2"].astype(f32)
    den_b2 = inp["den_b2"].astype(f32)
    ode_W1 = inp["ode_W1"].astype(f32)
    ode_b1 = inp["ode_b1"].astype(f32)
    ode_W2 = inp["ode_W2"].astype(f32)
    ode_b2 = inp["ode_b2"].astype(f32)
    ode_W3 = inp["ode_W3"].astype(f32)
    ode_b3 = inp["ode_b3"].astype(f32)
    dec_W = inp["dec_W"].astype(f32)
    dec_b = inp["dec_b"].astype(f32)

    sigma = float(np.log1p(np.exp(log_noise_std)))

    w31 = (dt_ * (ode_W3 @ ode_W1)).astype(f32)            # (U, U)
    w3g = (dt_ * (ode_W3 @ dec_W)).astype(f32)             # (U, 2)
    b3w1 = (ode_b3 @ ode_W1).astype(f32)                   # (U,)
    ts = np.arange(T, dtype=f32)
    bias1 = (ode_b1[:, None] + b3w1[:, None] * (ts[None, :] * dt_)).astype(f32)
    corr = (dt_ * (ode_b3 @ dec_W)).astype(f32)            # (2,)
    utri = np.triu(np.ones((CUM, CUM), dtype=f32))

    const_ins = {
        "projw": proj_W.reshape(NKT, 128, D).copy(),
        "projb": proj_b.reshape(D, 1).copy(),
        "lng": ln_g.reshape(D, 1).copy(),
        "lnb": ln_b.reshape(D, 1).copy(),
        "denw1": den_W1,
        "denb1": den_b1.reshape(U, 1).copy(),
        "denw2": den_W2,
        "denb2": den_b2.reshape(D, 1).copy(),
        "w1": ode_W1,
        "w2": ode_W2,
        "w31": w31,
        "w3g": w3g,
        "bias1": bias1,
        "b2c": ode_b2.reshape(U, 1).copy(),
        "corr": corr.reshape(2, 1).copy(),
        "decw": dec_W,
        "decb": dec_b.reshape(2, 1).copy(),
        "utri": utri,
        "ident": np.eye(128, dtype=f32),
        "invd": np.full((D, 1), 1.0 / D, dtype=f32),
        "onesd": np.ones((D, 1), dtype=f32),
        "ones1": np.ones((1, D), dtype=f32),
        "epsc": np.full((1, 1), LN_EPS, dtype=f32),
        "sigc": np.full((D, 1), sigma, dtype=f32),
    }
    const_ins = {k: np.ascontiguousarray(v, dtype=f32) for k, v in const_ins.items()}

    in_maps = []
    for c in range(N_CORES):
        m = dict(const_ins)
        m["bert"] = np.ascontiguousarray(bert_emb[c * BS:(c + 1) * BS])
        m["eps"] = np.ascontiguousarray(eps[c * BS:(c + 1) * BS])
        in_maps.append(m)

    from concourse.bass_utils import run_bass_kernel_spmd

    nc = _build(sigma)
    trace = os.environ.get("KERNEL_TRACE", "0") == "1"
    res = run_bass_kernel_spmd(
        nc, in_maps, core_ids=list(range(N_CORES)), trace=trace,
    )
    last_results = res

    pred = np.concatenate([res.results[c]["pred"] for c in range(N_CORES)], axis=0)
    stop = np.concatenate([res.results[c]["stp"] for c in range(N_CORES)], axis=0)
    loss_sum = np.sum([res.results[c]["loss"][0, 0] for c in range(N_CORES)])
    diff_loss = np.float32(loss_sum / (B * D))
    return pred, stop, diff_loss
